# revision 3
# baseline (speedup 1.0000x reference)
"""Bahdanau attention on 8 Trainium2 NeuronCores (Bass/Tile).

Problem (per reference):
  decoder_hidden (64, 512) f32, encoder_outputs (4096, 64, 512) f32,
  W1 (512,512), W2 (512,512), v (512,)
  dec_proj = decoder_hidden @ W1.T                       (B, H)
  enc_proj = einsum('bsh,gh->bsg', enc, W2)              (B, S, H)
  energy   = tanh(dec_proj[:,None,:] + enc_proj) @ v     (B, S)
  attn     = softmax(energy, axis=1)                     (B, S)
  context  = einsum('bs,bsh->bh', attn, enc)             (B, H)
  returns (context, attn)

Sharding: batch (64) split across 8 cores -> 8 batches/core. W1/W2/v
replicated. encoder_outputs is resharded host-side to (b, h, s) layout per
core so the contraction dim h lands on SBUF partitions with no on-chip
transposes; the kernel makes a single pass over the 64 MB/core stream.
"""

import numpy as np
import ml_dtypes

import concourse.bacc as bacc
import concourse.tile as tile
import concourse.mybir as mybir
from concourse.bass_utils import run_bass_kernel_spmd

F32 = mybir.dt.float32
BF16 = mybir.dt.bfloat16

NB = 8        # batches per core
H = 512
S = 4096
P = 128       # partitions
NCH = H // P  # h chunks (4)

TRACE = False
LAST_RESULTS = None


def build(nc, s_len):
    nt = s_len // P  # number of 128-wide s tiles per batch

    enc_d = nc.dram_tensor("enc", [NB, H, s_len], F32, kind="ExternalInput")
    dect_d = nc.dram_tensor("dect", [H, NB], F32, kind="ExternalInput")
    w1t_d = nc.dram_tensor("w1t", [H, H], F32, kind="ExternalInput")
    w2t_d = nc.dram_tensor("w2t", [H, H], F32, kind="ExternalInput")
    v_d = nc.dram_tensor("v2d", [1, H], F32, kind="ExternalInput")
    ctx_d = nc.dram_tensor("ctx_out", [NB, H], F32, kind="ExternalOutput")
    attn_d = nc.dram_tensor("attn_out", [NB, s_len], F32, kind="ExternalOutput")

    ident_np = np.eye(P, dtype=ml_dtypes.bfloat16)
    ident_d = nc.inline_tensor(np.asarray(ident_np), name="ident")
    ones_np = np.ones((1, P), dtype=ml_dtypes.bfloat16)
    ones_d = nc.inline_tensor(np.asarray(ones_np), name="ones1p")
    onescol_np = np.ones((P, 1), dtype=np.float32)
    onescol_d = nc.inline_tensor(onescol_np, name="onescol")

    # persistent SBUF tensors
    w2t_bf = nc.alloc_sbuf_tensor("w2t_bf", [P, NCH, H], BF16)
    v_bf = nc.alloc_sbuf_tensor("v_bf", [P, H], BF16)
    ident_sb = nc.alloc_sbuf_tensor("ident_sb", [P, P], BF16)
    ones_sb = nc.alloc_sbuf_tensor("ones_sb", [1, P], BF16)
    onescol_sb = nc.alloc_sbuf_tensor("onescol_sb", [P, 1], F32)
    dec_bf = nc.alloc_sbuf_tensor("dec_bf", [1, NB, H], BF16)
    w_all = nc.alloc_sbuf_tensor("w_all", [P, NB, nt], BF16)
    energy_all = nc.alloc_sbuf_tensor("energy_all", [P, NB, nt], F32)
    ctx_acc = nc.alloc_sbuf_tensor("ctx_acc", [P, NB, NCH], F32)

    with tile.TileContext(nc) as tc:
        # ---------------- prologue: constants + dec_proj ----------------
        with (
            tc.tile_pool(name="pro", bufs=1) as pro,
            tc.tile_pool(name="prop", bufs=2, space="PSUM") as prop,
        ):
            w2t_f = pro.tile([P, NCH, H], F32)
            nc.sync.dma_start(out=w2t_f[:], in_=w2t_d.ap().rearrange("(c p) g -> p c g", p=P))
            nc.scalar.copy(w2t_bf[:], w2t_f[:])

            v_f = pro.tile([1, H], F32)
            nc.sync.dma_start(out=v_f[:], in_=v_d[:])
            v_bf1 = pro.tile([1, H], BF16)
            nc.scalar.copy(v_bf1[:], v_f[:])
            nc.gpsimd.partition_broadcast(v_bf[:], v_bf1[:])

            nc.sync.dma_start(out=ident_sb[:], in_=ident_d[:])
            nc.sync.dma_start(out=ones_sb[:], in_=ones_d[:])
            nc.sync.dma_start(out=onescol_sb[:], in_=onescol_d[:])

            w1t_f = pro.tile([P, NCH, H], F32)
            nc.sync.dma_start(out=w1t_f[:], in_=w1t_d.ap().rearrange("(c p) g -> p c g", p=P))
            w1t_bf = pro.tile([P, NCH, H], BF16)
            nc.scalar.copy(w1t_bf[:], w1t_f[:])

            dect_f = pro.tile([P, NCH, NB], F32)
            nc.sync.dma_start(out=dect_f[:], in_=dect_d.ap().rearrange("(c p) b -> p c b", p=P))
            dect_bf = pro.tile([P, NCH, NB], BF16)
            nc.scalar.copy(dect_bf[:], dect_f[:])

            for b in range(NB):
                dp_ps = prop.tile([1, H], F32)
                for c in range(NCH):
                    nc.tensor.matmul(
                        dp_ps[:], dect_bf[:, c, b : b + 1], w1t_bf[:, c, :],
                        start=(c == 0), stop=(c == NCH - 1),
                    )
                nc.scalar.copy(dec_bf[:, b, :], dp_ps[:])

        # ---------------- main loop ----------------
        enc_r = [enc_d[b].rearrange("(c p) s -> p c s", p=P) for b in range(NB)]

        with (
            tc.tile_pool(name="pio", bufs=5) as pio,
            tc.tile_pool(name="pbf", bufs=8) as pbf,
            tc.tile_pool(name="ptan", bufs=4) as ptan,
            tc.tile_pool(name="pprod", bufs=3) as pprod,
            tc.tile_pool(name="pwt", bufs=4) as pwt,
            tc.tile_pool(name="pwb", bufs=4) as pwb,
            tc.tile_pool(name="psml", bufs=2) as psml,
            tc.tile_pool(name="ppP", bufs=3, space="PSUM") as ppP,
            tc.tile_pool(name="ppw", bufs=2, space="PSUM") as ppw,
            tc.tile_pool(name="ppb", bufs=2, space="PSUM") as ppb,
            tc.tile_pool(name="ppsm", bufs=1, space="PSUM") as ppsm,
        ):
            for b in range(NB):
                for t in range(nt):
                    tf = pio.tile([P, NCH, P], F32)
                    nc.sync.dma_start(out=tf[:], in_=enc_r[b][:, :, t * P : (t + 1) * P])
                    tb = pbf.tile([P, NCH, P], BF16)
                    nc.gpsimd.tensor_copy(tb[:], tf[:])

                    # P = enc @ W2T + dec_proj  -> psum [s=128, g=512]
                    p_ps = ppP.tile([P, H], F32)
                    for c in range(NCH):
                        nc.tensor.matmul(
                            p_ps[:], tb[:, c, :], w2t_bf[:, c, :],
                            start=(c == 0), stop=False,
                        )
                    nc.tensor.matmul(
                        p_ps[:], ones_sb[:], dec_bf[:, b, :],
                        start=False, stop=True,
                    )

                    tanh_t = ptan.tile([P, H], BF16)
                    nc.scalar.activation(tanh_t[:], p_ps[:], mybir.ActivationFunctionType.Tanh)

                    # energy[s] = sum_g tanh * v ; w = exp(energy)
                    prod_t = pprod.tile([P, H], BF16)
                    nc.vector.scalar_tensor_tensor(
                        out=prod_t[:], in0=tanh_t[:], scalar=1.0, in1=v_bf[:],
                        op0=mybir.AluOpType.mult, op1=mybir.AluOpType.mult,
                        accum_out=energy_all[:, b, t : t + 1],
                    )
                    nc.scalar.activation(
                        w_all[:, b, t : t + 1], energy_all[:, b, t : t + 1],
                        mybir.ActivationFunctionType.Exp,
                    )

                    # broadcast w over partitions: transpose then outer-product
                    wt_ps = ppw.tile([1, P], BF16)
                    nc.tensor.transpose(wt_ps[:], w_all[:, b, t : t + 1], ident_sb[:])
                    wt_sb = pwt.tile([1, P], BF16)
                    nc.scalar.copy(wt_sb[:], wt_ps[:])
                    wb_ps = ppb.tile([P, P], F32)
                    nc.tensor.matmul(wb_ps[:], ones_sb[:], wt_sb[:], start=True, stop=True)
                    wb_sb = pwb.tile([P, P], BF16)
                    nc.scalar.copy(wb_sb[:], wb_ps[:])

                    # ctx_acc[h] += sum_s w[s] * encT[h, s]
                    prod2 = pprod.tile([P, NCH, P], BF16, tag="prod2")
                    partial = pwt.tile([P, NCH], F32, tag="partial")
                    for c in range(NCH):
                        nc.vector.scalar_tensor_tensor(
                            out=prod2[:, c, :], in0=tb[:, c, :], scalar=1.0,
                            in1=wb_sb[:],
                            op0=mybir.AluOpType.mult, op1=mybir.AluOpType.mult,
                            accum_out=partial[:, c : c + 1],
                        )
                    if t == 0:
                        nc.vector.tensor_copy(ctx_acc[:, b, :], partial[:])
                    else:
                        nc.vector.tensor_tensor(
                            out=ctx_acc[:, b, :], in0=ctx_acc[:, b, :],
                            in1=partial[:], op=mybir.AluOpType.add,
                        )

                # ---- end of batch b: softmax normalize + outputs ----
                colsum = psml.tile([P, 1], F32, tag="colsum")
                nc.vector.reduce_sum(colsum[:], w_all[:, b, :], axis=mybir.AxisListType.X)
                sum_ps = ppsm.tile([1, 1], F32)
                nc.tensor.matmul(sum_ps[:], colsum[:], onescol_sb[:], start=True, stop=True)
                inv_sb = psml.tile([1, 1], F32, tag="inv")
                nc.vector.reciprocal(inv_sb[:], sum_ps[:])
                lninv = psml.tile([1, 1], F32, tag="lninv")
                nc.scalar.activation(lninv[:], inv_sb[:], mybir.ActivationFunctionType.Ln)
                lninv_b = psml.tile([P, 1], F32, tag="lninvb")
                nc.gpsimd.partition_broadcast(lninv_b[:], lninv[:])
                inv_b = psml.tile([P, 1], F32, tag="invb")
                nc.gpsimd.partition_broadcast(inv_b[:], inv_sb[:])

                attn_sb = psml.tile([P, nt], F32, tag="attn")
                nc.scalar.activation(
                    attn_sb[:], energy_all[:, b, :],
                    mybir.ActivationFunctionType.Exp, bias=lninv_b[:],
                )
                nc.sync.dma_start(
                    out=attn_d[b].rearrange("(t p) -> p t", p=P), in_=attn_sb[:]
                )

                ctx_sb = psml.tile([P, NCH], F32, tag="ctxo")
                nc.vector.tensor_scalar(
                    out=ctx_sb[:], in0=ctx_acc[:, b, :], scalar1=inv_b[:],
                    scalar2=None, op0=mybir.AluOpType.mult,
                )
                nc.sync.dma_start(
                    out=ctx_d[b].rearrange("(c p) -> p c", p=P), in_=ctx_sb[:]
                )

    return nc


_CACHE = {}


def _get_nc(s_len):
    if s_len not in _CACHE:
        nc = bacc.Bacc("TRN2", target_bir_lowering=False, debug=False)
        build(nc, s_len)
        nc.compile()
        _CACHE[s_len] = nc
    return _CACHE[s_len]


def _prep_inputs(decoder_hidden, encoder_outputs, W1, W2, v):
    """Host-side shard: batch across 8 cores; encT layout (b, h, s) per core."""
    s_len = encoder_outputs.shape[0]
    w1t = np.ascontiguousarray(np.asarray(W1, dtype=np.float32).T)
    w2t = np.ascontiguousarray(np.asarray(W2, dtype=np.float32).T)
    v2d = np.ascontiguousarray(np.asarray(v, dtype=np.float32).reshape(1, H))
    enc = np.asarray(encoder_outputs, dtype=np.float32)
    dec = np.asarray(decoder_hidden, dtype=np.float32)
    in_maps = []
    for c in range(8):
        bsl = slice(c * NB, (c + 1) * NB)
        enc_c = np.ascontiguousarray(enc[:, bsl, :].transpose(1, 2, 0))
        dect_c = np.ascontiguousarray(dec[bsl, :].T)
        in_maps.append(
            {"enc": enc_c, "dect": dect_c, "w1t": w1t, "w2t": w2t, "v2d": v2d}
        )
    return in_maps, s_len


def kernel(decoder_hidden, encoder_outputs, W1, W2, v):
    global LAST_RESULTS
    in_maps, s_len = _prep_inputs(decoder_hidden, encoder_outputs, W1, W2, v)
    nc = _get_nc(s_len)
    res = run_bass_kernel_spmd(nc, in_maps, core_ids=list(range(8)), trace=TRACE)
    LAST_RESULTS = res
    B = 8 * NB
    context = np.empty((B, H), dtype=np.float32)
    attn = np.empty((B, s_len), dtype=np.float32)
    for c in range(8):
        bsl = slice(c * NB, (c + 1) * NB)
        context[bsl] = res.results[c]["ctx_out"]
        attn[bsl] = res.results[c]["attn_out"]
    return (context, attn)


# revision 4
# speedup vs baseline: 1.4789x; 1.4789x over previous
"""Bahdanau attention on 8 Trainium2 NeuronCores (Bass/Tile).

Problem (per reference):
  decoder_hidden (64, 512) f32, encoder_outputs (4096, 64, 512) f32,
  W1 (512,512), W2 (512,512), v (512,)
  dec_proj = decoder_hidden @ W1.T                       (B, H)
  enc_proj = einsum('bsh,gh->bsg', enc, W2)              (B, S, H)
  energy   = tanh(dec_proj[:,None,:] + enc_proj) @ v     (B, S)
  attn     = softmax(energy, axis=1)                     (B, S)
  context  = einsum('bs,bsh->bh', attn, enc)             (B, H)
  returns (context, attn)

Sharding: batch (64) split across 8 cores -> 8 batches/core; W1/W2/v
replicated. encoder_outputs is resharded host-side to (b, h, s) layout per
core so the contraction dim h lands on SBUF partitions; the kernel makes a
single pass over the 64 MB/core stream.

Per 512-column s-tile (one DMA of [128p, 4hc, 512s] f32, 2 KB rows):
  cast f32->bf16 (DVE)
  PT[g,s] = W2T-chunk.T @ encT-chunk   16 matmuls into one 4-bank PSUM tile
  tanh(PT + dec_projT[g,b])            4 ACT ops, bias folded per-partition
  energy = v.T @ tanhPT                4 matmuls -> psum [1, 512]
  w = exp(energy) (+ running sum via ACT accum), cast w bf16 (DVE)
  broadcast w across partitions (GPSIMD)
  ctx partials += encT * w             4 DVE scalar_tensor_tensor accums
End of batch: reduce partials, softmax-normalize, DMA outputs.
"""

import numpy as np
import ml_dtypes

import concourse.bacc as bacc
import concourse.tile as tile
import concourse.mybir as mybir
from concourse.bass_utils import run_bass_kernel_spmd

F32 = mybir.dt.float32
BF16 = mybir.dt.bfloat16
AF = mybir.ActivationFunctionType

NB = 8         # batches per core
H = 512
P = 128        # partitions
NCH = H // P   # h chunks (4)
TS = 512       # s columns per tile

TRACE = False
LAST_RESULTS = None


def build(nc, s_len):
    nt = s_len // TS  # s tiles per batch

    enc_d = nc.dram_tensor("enc", [NB, H, s_len], F32, kind="ExternalInput")
    dect_d = nc.dram_tensor("dect", [H, NB], F32, kind="ExternalInput")
    w1t_d = nc.dram_tensor("w1t", [H, H], F32, kind="ExternalInput")
    w2t_d = nc.dram_tensor("w2t", [H, H], F32, kind="ExternalInput")
    v_d = nc.dram_tensor("v2d", [1, H], F32, kind="ExternalInput")
    ctx_d = nc.dram_tensor("ctx_out", [NB, H], F32, kind="ExternalOutput")
    attn_d = nc.dram_tensor("attn_out", [NB, s_len], F32, kind="ExternalOutput")

    # persistent SBUF
    w2t_bf = nc.alloc_sbuf_tensor("w2t_bf", [P, NCH, H], BF16)   # [h, hc, g]
    v_sb = nc.alloc_sbuf_tensor("v_sb", [P, NCH], BF16)          # v chunks [g, gc]
    dpt_sb = nc.alloc_sbuf_tensor("dpt_sb", [P, NCH, NB], F32)   # dec_projT [g, gc, b]
    wsum = nc.alloc_sbuf_tensor("wsum", [1, NB, nt], F32)

    with tile.TileContext(nc) as tc:
        # ---------------- prologue ----------------
        with (
            tc.tile_pool(name="pro", bufs=1) as pro,
            tc.tile_pool(name="prop", bufs=1, space="PSUM") as prop,
        ):
            w2t_f = pro.tile([P, NCH, H], F32)
            nc.sync.dma_start(out=w2t_f[:], in_=w2t_d.ap().rearrange("(c p) g -> p c g", p=P))
            nc.scalar.copy(w2t_bf[:], w2t_f[:])

            v_f = pro.tile([P, NCH], F32)
            # v[g] -> [g % 128, g // 128]
            nc.sync.dma_start(out=v_f[:], in_=v_d.ap().rearrange("o (c p) -> p (o c)", p=P))
            nc.vector.tensor_copy(v_sb[:], v_f[:])

            w1t_f = pro.tile([P, NCH, H], F32)
            nc.sync.dma_start(out=w1t_f[:], in_=w1t_d.ap().rearrange("(c p) g -> p c g", p=P))
            w1t_bf = pro.tile([P, NCH, H], BF16)
            nc.scalar.copy(w1t_bf[:], w1t_f[:])

            dect_f = pro.tile([P, NCH, NB], F32)
            nc.sync.dma_start(out=dect_f[:], in_=dect_d.ap().rearrange("(c p) b -> p c b", p=P))
            dect_bf = pro.tile([P, NCH, NB], BF16)
            nc.scalar.copy(dect_bf[:], dect_f[:])

            # dec_projT[g, b] = sum_h2 W1[g, h2] dec[b, h2]
            dp_ps = prop.tile([P, NCH, NB], F32)
            for gc in range(NCH):
                for hc in range(NCH):
                    nc.tensor.matmul(
                        dp_ps[:, gc, :],
                        w1t_bf[:, hc, gc * P : (gc + 1) * P],
                        dect_bf[:, hc, :],
                        start=(hc == 0), stop=(hc == NCH - 1),
                    )
            nc.scalar.copy(dpt_sb[:], dp_ps[:])

        # ---------------- main loop ----------------
        enc_r = [enc_d[b].rearrange("(c p) s -> p c s", p=P) for b in range(NB)]

        with (
            tc.tile_pool(name="pio", bufs=3) as pio,
            tc.tile_pool(name="pbf", bufs=4) as pbf,
            tc.tile_pool(name="ptan", bufs=2) as ptan,
            tc.tile_pool(name="pw", bufs=3) as pw,
            tc.tile_pool(name="pwb", bufs=3) as pwb,
            tc.tile_pool(name="ppart", bufs=2) as ppart,
            tc.tile_pool(name="psml", bufs=2) as psml,
            tc.tile_pool(name="prow", bufs=2) as prow,
            tc.tile_pool(name="ppP", bufs=1, space="PSUM") as ppP,
            tc.tile_pool(name="ppe", bufs=2, space="PSUM") as ppe,
        ):
            for b in range(NB):
                part_t = ppart.tile([P, NCH, nt], F32)
                w_row = prow.tile([1, nt, TS], F32, tag="wrow")
                for t in range(nt):
                    tf = pio.tile([P, NCH, TS], F32)
                    nc.sync.dma_start(out=tf[:], in_=enc_r[b][:, :, t * TS : (t + 1) * TS])
                    tb = pbf.tile([P, NCH, TS], BF16)
                    nc.vector.tensor_copy(tb[:], tf[:])

                    # PT[g, s] = sum_h W2[g, h] enc[h, s]; 4 psum banks (gc)
                    pt_ps = ppP.tile([P, NCH, TS], F32)
                    for gc in range(NCH):
                        for hc in range(NCH):
                            nc.tensor.matmul(
                                pt_ps[:, gc, :],
                                w2t_bf[:, hc, gc * P : (gc + 1) * P],
                                tb[:, hc, :],
                                start=(hc == 0), stop=(hc == NCH - 1),
                            )

                    # tanh(PT + dec_projT[g, b]) per gc, bias per-partition
                    tanh_t = ptan.tile([P, NCH, TS], BF16)
                    for gc in range(NCH):
                        nc.scalar.activation(
                            tanh_t[:, gc, :], pt_ps[:, gc, :], AF.Tanh,
                            bias=dpt_sb[:, gc, b : b + 1],
                        )

                    # energy[1, s] = sum_g v[g] tanhPT[g, s]
                    e_ps = ppe.tile([1, TS], F32)
                    for gc in range(NCH):
                        nc.tensor.matmul(
                            e_ps[:], v_sb[:, gc : gc + 1], tanh_t[:, gc, :],
                            start=(gc == 0), stop=(gc == NCH - 1),
                        )

                    # w = exp(energy); accumulate sum(w) as side output
                    nc.scalar.activation(
                        w_row[:, t, :], e_ps[:], AF.Exp,
                        accum_out=wsum[:, b, t : t + 1],
                    )
                    w_bf = pw.tile([1, TS], BF16)
                    nc.vector.tensor_copy(w_bf[:], w_row[:, t, :])
                    wb_sb = pwb.tile([P, TS], BF16)
                    nc.gpsimd.partition_broadcast(wb_sb[:], w_bf[:])

                    # ctx partials[h, hc] = sum_s enc[h, s] w[s]
                    for hc in range(NCH):
                        nc.vector.scalar_tensor_tensor(
                            out=tb[:, hc, :], in0=tb[:, hc, :], scalar=1.0,
                            in1=wb_sb[:],
                            op0=mybir.AluOpType.mult, op1=mybir.AluOpType.mult,
                            accum_out=part_t[:, hc, t : t + 1],
                        )

                # ---- end of batch b ----
                bsum = psml.tile([1, 1], F32, tag="bsum")
                nc.vector.reduce_sum(bsum[:], wsum[:, b, :], axis=mybir.AxisListType.X)
                inv_sb = psml.tile([1, 1], F32, tag="inv")
                nc.vector.reciprocal(inv_sb[:], bsum[:])

                attn_row = prow.tile([1, nt * TS], F32, tag="attnrow")
                nc.vector.tensor_scalar(
                    out=attn_row[:], in0=w_row[0:1, :, :].rearrange("p a b -> p (a b)"),
                    scalar1=inv_sb[:], scalar2=None, op0=mybir.AluOpType.mult,
                )
                nc.sync.dma_start(out=attn_d[b : b + 1, :], in_=attn_row[:])

                inv_b = psml.tile([P, 1], F32, tag="invb")
                nc.gpsimd.partition_broadcast(inv_b[:], inv_sb[:])
                ctx_red = psml.tile([P, NCH], F32, tag="ctxred")
                nc.vector.reduce_sum(ctx_red[:], part_t[:], axis=mybir.AxisListType.X)
                ctx_sb = psml.tile([P, NCH], F32, tag="ctxo")
                nc.vector.tensor_scalar(
                    out=ctx_sb[:], in0=ctx_red[:], scalar1=inv_b[:],
                    scalar2=None, op0=mybir.AluOpType.mult,
                )
                nc.sync.dma_start(out=ctx_d[b].rearrange("(c p) -> p c", p=P), in_=ctx_sb[:])

    return nc


_CACHE = {}


def _get_nc(s_len):
    if s_len not in _CACHE:
        nc = bacc.Bacc("TRN2", target_bir_lowering=False, debug=False)
        build(nc, s_len)
        nc.compile()
        _CACHE[s_len] = nc
    return _CACHE[s_len]


def _prep_inputs(decoder_hidden, encoder_outputs, W1, W2, v):
    """Host-side shard: batch across 8 cores; encT layout (b, h, s) per core."""
    s_len = encoder_outputs.shape[0]
    w1t = np.ascontiguousarray(np.asarray(W1, dtype=np.float32).T)
    w2t = np.ascontiguousarray(np.asarray(W2, dtype=np.float32).T)
    v2d = np.ascontiguousarray(np.asarray(v, dtype=np.float32).reshape(1, H))
    enc = np.asarray(encoder_outputs, dtype=np.float32)
    dec = np.asarray(decoder_hidden, dtype=np.float32)
    in_maps = []
    for c in range(8):
        bsl = slice(c * NB, (c + 1) * NB)
        enc_c = np.ascontiguousarray(enc[:, bsl, :].transpose(1, 2, 0))
        dect_c = np.ascontiguousarray(dec[bsl, :].T)
        in_maps.append(
            {"enc": enc_c, "dect": dect_c, "w1t": w1t, "w2t": w2t, "v2d": v2d}
        )
    return in_maps, s_len


def kernel(decoder_hidden, encoder_outputs, W1, W2, v):
    global LAST_RESULTS
    in_maps, s_len = _prep_inputs(decoder_hidden, encoder_outputs, W1, W2, v)
    nc = _get_nc(s_len)
    res = run_bass_kernel_spmd(nc, in_maps, core_ids=list(range(8)), trace=TRACE)
    LAST_RESULTS = res
    B = 8 * NB
    context = np.empty((B, H), dtype=np.float32)
    attn = np.empty((B, s_len), dtype=np.float32)
    for c in range(8):
        bsl = slice(c * NB, (c + 1) * NB)
        context[bsl] = res.results[c]["ctx_out"]
        attn[bsl] = res.results[c]["attn_out"]
    return (context, attn)


# revision 8
# speedup vs baseline: 1.5702x; 1.0617x over previous
"""Bahdanau attention on 8 Trainium2 NeuronCores (Bass/Tile).

Problem (per reference):
  decoder_hidden (64, 512) f32, encoder_outputs (4096, 64, 512) f32,
  W1 (512,512), W2 (512,512), v (512,)
  dec_proj = decoder_hidden @ W1.T                       (B, H)
  enc_proj = einsum('bsh,gh->bsg', enc, W2)              (B, S, H)
  energy   = tanh(dec_proj[:,None,:] + enc_proj) @ v     (B, S)
  attn     = softmax(energy, axis=1)                     (B, S)
  context  = einsum('bs,bsh->bh', attn, enc)             (B, H)
  returns (context, attn)

Sharding: batch (64) split across 8 cores -> 8 batches/core; W1/W2/v
replicated. encoder_outputs is resharded host-side to (b, h, s) layout per
core so the contraction dim h lands on SBUF partitions; the kernel makes a
single pass over the 64 MB/core stream.

Per 512-column s-tile (one DMA of [128p, 4hc, 512s] f32, 2 KB rows):
  cast f32->bf16 (DVE)
  PT[g,s] = W2T-chunk.T @ encT-chunk   16 matmuls into one 4-bank PSUM tile
  tanh(PT + dec_projT[g,b])            4 ACT ops, bias folded per-partition
  energy = v.T @ tanhPT                4 matmuls -> psum [1, 512]
  w = exp(energy) (+ running sum via ACT accum), cast w bf16 (DVE)
  broadcast w across partitions (GPSIMD)
  ctx partials += encT * w             4 DVE scalar_tensor_tensor accums
End of batch: reduce partials, softmax-normalize, DMA outputs.
"""

import numpy as np
import ml_dtypes

import concourse.bacc as bacc
import concourse.tile as tile
import concourse.mybir as mybir
from concourse.bass_utils import run_bass_kernel_spmd

F32 = mybir.dt.float32
BF16 = mybir.dt.bfloat16
AF = mybir.ActivationFunctionType

NB = 8         # batches per core
H = 512
P = 128        # partitions
NCH = H // P   # h chunks (4)
TS = 512       # s columns per tile

TRACE = False
LAST_RESULTS = None


def build(nc, s_len):
    nt = s_len // TS  # s tiles per batch

    enc_d = nc.dram_tensor("enc", [NB, H, s_len], F32, kind="ExternalInput")
    dect_d = nc.dram_tensor("dect", [H, NB], F32, kind="ExternalInput")
    w1t_d = nc.dram_tensor("w1t", [H, H], F32, kind="ExternalInput")
    w2t_d = nc.dram_tensor("w2t", [H, H], F32, kind="ExternalInput")
    v_d = nc.dram_tensor("v2d", [1, H], F32, kind="ExternalInput")
    ctx_d = nc.dram_tensor("ctx_out", [NB, H], F32, kind="ExternalOutput")
    attn_d = nc.dram_tensor("attn_out", [NB, s_len], F32, kind="ExternalOutput")

    # persistent SBUF
    w2t_bf = nc.alloc_sbuf_tensor("w2t_bf", [P, NCH, H], BF16)   # [h, hc, g]
    v_sb = nc.alloc_sbuf_tensor("v_sb", [P, NCH], BF16)          # v chunks [g, gc]
    dpt_sb = nc.alloc_sbuf_tensor("dpt_sb", [P, NCH, NB], F32)   # dec_projT [g, gc, b]
    wsum = nc.alloc_sbuf_tensor("wsum", [1, NB, nt], F32)

    with tile.TileContext(nc) as tc:
        # ---------------- prologue ----------------
        with (
            tc.tile_pool(name="pro", bufs=1) as pro,
            tc.tile_pool(name="prop", bufs=1, space="PSUM") as prop,
        ):
            w2t_f = pro.tile([P, NCH, H], F32)
            nc.sync.dma_start(out=w2t_f[:], in_=w2t_d.ap().rearrange("(c p) g -> p c g", p=P))
            nc.scalar.copy(w2t_bf[:], w2t_f[:])

            v_f = pro.tile([P, NCH], F32)
            # v[g] -> [g % 128, g // 128]
            nc.sync.dma_start(out=v_f[:], in_=v_d.ap().rearrange("o (c p) -> p (o c)", p=P))
            nc.vector.tensor_copy(v_sb[:], v_f[:])

            w1t_f = pro.tile([P, NCH, H], F32)
            nc.sync.dma_start(out=w1t_f[:], in_=w1t_d.ap().rearrange("(c p) g -> p c g", p=P))
            w1t_bf = pro.tile([P, NCH, H], BF16)
            nc.scalar.copy(w1t_bf[:], w1t_f[:])

            dect_f = pro.tile([P, NCH, NB], F32)
            nc.sync.dma_start(out=dect_f[:], in_=dect_d.ap().rearrange("(c p) b -> p c b", p=P))
            dect_bf = pro.tile([P, NCH, NB], BF16)
            nc.scalar.copy(dect_bf[:], dect_f[:])

            # dec_projT[g, b] = sum_h2 W1[g, h2] dec[b, h2]
            dp_ps = prop.tile([P, NCH, NB], F32)
            for gc in range(NCH):
                for hc in range(NCH):
                    nc.tensor.matmul(
                        dp_ps[:, gc, :],
                        w1t_bf[:, hc, gc * P : (gc + 1) * P],
                        dect_bf[:, hc, :],
                        start=(hc == 0), stop=(hc == NCH - 1),
                    )
            nc.scalar.copy(dpt_sb[:], dp_ps[:])

        # ---------------- main loop (software-pipelined emission) ----------------
        enc_r = [enc_d[b].rearrange("(c p) s -> p c s", p=P) for b in range(NB)]
        G = min(4, nt)  # energy col-pack group size
        assert nt % G == 0
        total = NB * nt

        with (
            tc.tile_pool(name="pio", bufs=4) as pio,
            tc.tile_pool(name="pbf", bufs=12) as pbf,
            tc.tile_pool(name="ptan", bufs=10) as ptan,
            tc.tile_pool(name="pw", bufs=4) as pw,
            tc.tile_pool(name="pwb", bufs=4) as pwb,
            tc.tile_pool(name="ppart", bufs=2) as ppart,
            tc.tile_pool(name="psml", bufs=2) as psml,
            tc.tile_pool(name="prow", bufs=2) as prow,
            tc.tile_pool(name="ppP", bufs=1, space="PSUM") as ppP,
            tc.tile_pool(name="ppe", bufs=2, space="PSUM") as ppe,
        ):
            tbs = {}      # k -> bf16 enc tile
            tanhs = {}    # k -> tanh tile
            e4s = {}      # group -> packed energy psum [128, TS]
            part_ts = {}  # b -> ctx partials
            w_rows = {}   # b -> w rows

            def front(k):
                """DMA + cast + P-matmuls + tanh for tile k."""
                b, t = divmod(k, nt)
                if t == 0:
                    part_ts[b] = ppart.tile([P, NCH, nt], F32, tag="part", name="part")
                    w_rows[b] = prow.tile([1, nt, TS], F32, tag="wrow", name="wrow")
                tf = pio.tile([P, NCH, TS], F32)
                nc.sync.dma_start(out=tf[:], in_=enc_r[b][:, :, t * TS : (t + 1) * TS])
                tb = pbf.tile([P, NCH, TS], BF16)
                nc.vector.tensor_copy(tb[:], tf[:])
                tbs[k] = tb

                pt_ps = ppP.tile([P, NCH, TS], F32)
                for gc in range(NCH):
                    for hc in range(NCH):
                        nc.tensor.matmul(
                            pt_ps[:, gc, :],
                            w2t_bf[:, hc, gc * P : (gc + 1) * P],
                            tb[:, hc, :],
                            start=(hc == 0), stop=(hc == NCH - 1),
                        )

                tanh_t = ptan.tile([P, NCH, TS], BF16)
                for gc in range(NCH):
                    nc.scalar.activation(
                        tanh_t[:, gc, :], pt_ps[:, gc, :], AF.Tanh,
                        bias=dpt_sb[:, gc, b : b + 1],
                    )
                tanhs[k] = tanh_t

            def energy_group(g):
                """Col-packed energy matmuls for tiles 4g..4g+3 (one batch)."""
                e4 = ppe.tile([P, TS], F32, tag="e4", name="e4")
                for gc in range(NCH):
                    for j in range(G):
                        k = g * G + j
                        nc.tensor.matmul(
                            e4[32 * j : 32 * j + 1, :],
                            v_sb[:, gc : gc + 1], tanhs[k][:, gc, :],
                            start=(gc == 0), stop=(gc == NCH - 1),
                            tile_position=(0, 32 * j),
                        )
                e4s[g] = e4

            def back(k):
                """exp + broadcast + ctx accumulation for tile k."""
                b, t = divmod(k, nt)
                g, j = divmod(k, G)
                e4 = e4s[g]
                w_row = w_rows[b]
                nc.scalar.activation(
                    w_row[:, t, :], e4[32 * j : 32 * j + 1, :], AF.Exp,
                    accum_out=wsum[:, b, t : t + 1],
                )
                w_bf = pw.tile([1, TS], BF16)
                nc.vector.tensor_copy(w_bf[:], w_row[:, t, :])
                wb_sb = pwb.tile([P, TS], BF16)
                nc.gpsimd.partition_broadcast(wb_sb[:], w_bf[:])

                tb = tbs.pop(k)
                for hc in range(NCH):
                    nc.vector.scalar_tensor_tensor(
                        out=tb[:, hc, :], in0=tb[:, hc, :], scalar=1.0,
                        in1=wb_sb[:],
                        op0=mybir.AluOpType.mult, op1=mybir.AluOpType.mult,
                        accum_out=part_ts[b][:, hc, t : t + 1],
                    )
                tanhs.pop(k, None)

            def endb(b):
                part_t = part_ts.pop(b)
                w_row = w_rows.pop(b)
                bsum = psml.tile([1, 1], F32, tag="bsum")
                nc.vector.reduce_sum(bsum[:], wsum[:, b, :], axis=mybir.AxisListType.X)
                inv_sb = psml.tile([1, 1], F32, tag="inv")
                nc.vector.reciprocal(inv_sb[:], bsum[:])

                attn_row = prow.tile([1, nt * TS], F32, tag="attnrow")
                nc.vector.tensor_scalar(
                    out=attn_row[:], in0=w_row[0:1, :, :].rearrange("p a b -> p (a b)"),
                    scalar1=inv_sb[:], scalar2=None, op0=mybir.AluOpType.mult,
                )
                nc.sync.dma_start(out=attn_d[b : b + 1, :], in_=attn_row[:])

                inv_b = psml.tile([P, 1], F32, tag="invb")
                nc.gpsimd.partition_broadcast(inv_b[:], inv_sb[:])
                ctx_red = psml.tile([P, NCH], F32, tag="ctxred")
                nc.vector.reduce_sum(ctx_red[:], part_t[:], axis=mybir.AxisListType.X)
                ctx_sb = psml.tile([P, NCH], F32, tag="ctxo")
                nc.vector.tensor_scalar(
                    out=ctx_sb[:], in0=ctx_red[:], scalar1=inv_b[:],
                    scalar2=None, op0=mybir.AluOpType.mult,
                )
                nc.sync.dma_start(out=ctx_d[b].rearrange("(c p) -> p c", p=P), in_=ctx_sb[:])

            # pipeline: energy group g is emitted after the P-matmuls of
            # group g+1, so PE never stalls on the tanh of its own group.
            ngroups = total // G
            for k in range(total):
                front(k)
                if k % G == G - 1:
                    g_now = k // G
                    if g_now >= 1:
                        g = g_now - 1
                        energy_group(g)
                        for kk in range(g * G, (g + 1) * G):
                            back(kk)
                        if (g + 1) * G % nt == 0:
                            endb((g * G) // nt)
            # drain last group
            g = ngroups - 1
            energy_group(g)
            for kk in range(g * G, (g + 1) * G):
                back(kk)
            endb(NB - 1)

    return nc


_CACHE = {}


def _get_nc(s_len):
    if s_len not in _CACHE:
        nc = bacc.Bacc("TRN2", target_bir_lowering=False, debug=False)
        build(nc, s_len)
        nc.compile()
        _CACHE[s_len] = nc
    return _CACHE[s_len]


def _prep_inputs(decoder_hidden, encoder_outputs, W1, W2, v):
    """Host-side shard: batch across 8 cores; encT layout (b, h, s) per core."""
    s_len = encoder_outputs.shape[0]
    w1t = np.ascontiguousarray(np.asarray(W1, dtype=np.float32).T)
    w2t = np.ascontiguousarray(np.asarray(W2, dtype=np.float32).T)
    v2d = np.ascontiguousarray(np.asarray(v, dtype=np.float32).reshape(1, H))
    enc = np.asarray(encoder_outputs, dtype=np.float32)
    dec = np.asarray(decoder_hidden, dtype=np.float32)
    in_maps = []
    for c in range(8):
        bsl = slice(c * NB, (c + 1) * NB)
        enc_c = np.ascontiguousarray(enc[:, bsl, :].transpose(1, 2, 0))
        dect_c = np.ascontiguousarray(dec[bsl, :].T)
        in_maps.append(
            {"enc": enc_c, "dect": dect_c, "w1t": w1t, "w2t": w2t, "v2d": v2d}
        )
    return in_maps, s_len


def kernel(decoder_hidden, encoder_outputs, W1, W2, v):
    global LAST_RESULTS
    in_maps, s_len = _prep_inputs(decoder_hidden, encoder_outputs, W1, W2, v)
    nc = _get_nc(s_len)
    res = run_bass_kernel_spmd(nc, in_maps, core_ids=list(range(8)), trace=TRACE)
    LAST_RESULTS = res
    B = 8 * NB
    context = np.empty((B, H), dtype=np.float32)
    attn = np.empty((B, s_len), dtype=np.float32)
    for c in range(8):
        bsl = slice(c * NB, (c + 1) * NB)
        context[bsl] = res.results[c]["ctx_out"]
        attn[bsl] = res.results[c]["attn_out"]
    return (context, attn)


# revision 12
# speedup vs baseline: 1.5836x; 1.0086x over previous
"""Bahdanau attention on 8 Trainium2 NeuronCores (Bass/Tile).

Problem (per reference):
  decoder_hidden (64, 512) f32, encoder_outputs (4096, 64, 512) f32,
  W1 (512,512), W2 (512,512), v (512,)
  dec_proj = decoder_hidden @ W1.T                       (B, H)
  enc_proj = einsum('bsh,gh->bsg', enc, W2)              (B, S, H)
  energy   = tanh(dec_proj[:,None,:] + enc_proj) @ v     (B, S)
  attn     = softmax(energy, axis=1)                     (B, S)
  context  = einsum('bs,bsh->bh', attn, enc)             (B, H)
  returns (context, attn)

Sharding: batch (64) split across 8 cores -> 8 batches/core; W1/W2/v
replicated. encoder_outputs is resharded host-side to (b, h, s) layout per
core so the contraction dim h lands on SBUF partitions; the kernel makes a
single pass over the 64 MB/core stream.

Per 512-column s-tile (one DMA of [128p, 4hc, 512s] f32, 2 KB rows):
  cast f32->bf16 (DVE)
  PT[g,s] = W2T-chunk.T @ encT-chunk   16 matmuls into one 4-bank PSUM tile
  tanh(PT + dec_projT[g,b])            4 ACT ops, bias folded per-partition
  energy = v.T @ tanhPT                4 matmuls -> psum [1, 512]
  w = exp(energy) (+ running sum via ACT accum), cast w bf16 (DVE)
  broadcast w across partitions (GPSIMD)
  ctx partials += encT * w             4 DVE scalar_tensor_tensor accums
End of batch: reduce partials, softmax-normalize, DMA outputs.
"""

import numpy as np
import ml_dtypes

import concourse.bacc as bacc
import concourse.tile as tile
import concourse.mybir as mybir
from concourse.bass_utils import run_bass_kernel_spmd

F32 = mybir.dt.float32
BF16 = mybir.dt.bfloat16
AF = mybir.ActivationFunctionType

NB = 8         # batches per core
H = 512
P = 128        # partitions
NCH = H // P   # h chunks (4)
TS = 512       # s columns per tile

TRACE = False
LAST_RESULTS = None


def build(nc, s_len):
    nt = s_len // TS  # s tiles per batch

    enc_d = nc.dram_tensor("enc", [NB, H, s_len], F32, kind="ExternalInput")
    dect_d = nc.dram_tensor("dect", [H, NB], F32, kind="ExternalInput")
    w1t_d = nc.dram_tensor("w1t", [H, H], F32, kind="ExternalInput")
    w2t_d = nc.dram_tensor("w2t", [H, H], F32, kind="ExternalInput")
    v_d = nc.dram_tensor("v2d", [1, H], F32, kind="ExternalInput")
    ctx_d = nc.dram_tensor("ctx_out", [NB, H], F32, kind="ExternalOutput")
    attn_d = nc.dram_tensor("attn_out", [NB, s_len], F32, kind="ExternalOutput")

    # persistent SBUF
    w2t_bf = nc.alloc_sbuf_tensor("w2t_bf", [P, NCH, H], BF16)   # [h, hc, g]
    v_sb = nc.alloc_sbuf_tensor("v_sb", [P, NCH], BF16)          # v chunks [g, gc]
    dpt_sb = nc.alloc_sbuf_tensor("dpt_sb", [P, NCH, NB], F32)   # dec_projT [g, gc, b]
    wsum = nc.alloc_sbuf_tensor("wsum", [1, NB, nt], F32)

    with tile.TileContext(nc) as tc:
        # ---------------- prologue ----------------
        with (
            tc.tile_pool(name="pro", bufs=1) as pro,
            tc.tile_pool(name="prop", bufs=1, space="PSUM") as prop,
        ):
            w2t_f = pro.tile([P, NCH, H], F32)
            nc.sync.dma_start(out=w2t_f[:], in_=w2t_d.ap().rearrange("(c p) g -> p c g", p=P))
            nc.scalar.copy(w2t_bf[:], w2t_f[:])

            v_f = pro.tile([P, NCH], F32)
            # v[g] -> [g % 128, g // 128]
            nc.sync.dma_start(out=v_f[:], in_=v_d.ap().rearrange("o (c p) -> p (o c)", p=P))
            nc.vector.tensor_copy(v_sb[:], v_f[:])

            w1t_f = pro.tile([P, NCH, H], F32)
            nc.sync.dma_start(out=w1t_f[:], in_=w1t_d.ap().rearrange("(c p) g -> p c g", p=P))
            w1t_bf = pro.tile([P, NCH, H], BF16)
            nc.scalar.copy(w1t_bf[:], w1t_f[:])

            dect_f = pro.tile([P, NCH, NB], F32)
            nc.sync.dma_start(out=dect_f[:], in_=dect_d.ap().rearrange("(c p) b -> p c b", p=P))
            dect_bf = pro.tile([P, NCH, NB], BF16)
            nc.scalar.copy(dect_bf[:], dect_f[:])

            # dec_projT[g, b] = sum_h2 W1[g, h2] dec[b, h2]
            dp_ps = prop.tile([P, NCH, NB], F32)
            for gc in range(NCH):
                for hc in range(NCH):
                    nc.tensor.matmul(
                        dp_ps[:, gc, :],
                        w1t_bf[:, hc, gc * P : (gc + 1) * P],
                        dect_bf[:, hc, :],
                        start=(hc == 0), stop=(hc == NCH - 1),
                    )
            nc.scalar.copy(dpt_sb[:], dp_ps[:])

        # ---------------- main loop (software-pipelined emission) ----------------
        enc_r = [enc_d[b].rearrange("(c p) s -> p c s", p=P) for b in range(NB)]
        G = min(4, nt)  # energy col-pack group size
        assert nt % G == 0
        total = NB * nt

        with (
            tc.tile_pool(name="pio", bufs=7) as pio,
            tc.tile_pool(name="pbf", bufs=14) as pbf,
            tc.tile_pool(name="ptan", bufs=10) as ptan,
            tc.tile_pool(name="pw", bufs=4) as pw,
            tc.tile_pool(name="pwb", bufs=4) as pwb,
            tc.tile_pool(name="ppart", bufs=2) as ppart,
            tc.tile_pool(name="psml", bufs=2) as psml,
            tc.tile_pool(name="prow", bufs=2) as prow,
            tc.tile_pool(name="ppP", bufs=1, space="PSUM") as ppP,
            tc.tile_pool(name="ppe", bufs=2, space="PSUM") as ppe,
        ):
            tfs = {}      # k -> f32 enc tile
            tbs = {}      # k -> bf16 enc tile
            tanhs = {}    # k -> tanh tile
            e4s = {}      # group -> packed energy psum [128, TS]
            part_ts = {}  # b -> ctx partials
            w_rows = {}   # b -> w rows

            def loadcast(k):
                """DMA + cast for tile k (keeps DVE casts ahead of ctx blocks)."""
                b, t = divmod(k, nt)
                if t == 0:
                    part_ts[b] = ppart.tile([P, NCH, nt], F32, tag="part", name="part")
                    w_rows[b] = prow.tile([1, nt, TS], F32, tag="wrow", name="wrow")
                tf = pio.tile([P, NCH, TS], F32)
                nc.sync.dma_start(out=tf[:], in_=enc_r[b][:, :, t * TS : (t + 1) * TS])
                tb = pbf.tile([P, NCH, TS], BF16)
                nc.vector.tensor_copy(tb[:], tf[:])
                tfs[k] = tf
                tbs[k] = tb

            def pmmtanh(k):
                b, t = divmod(k, nt)
                tb = tbs[k]
                pt_ps = ppP.tile([P, NCH, TS], F32)
                for gc in range(NCH):
                    for hc in range(NCH):
                        nc.tensor.matmul(
                            pt_ps[:, gc, :],
                            w2t_bf[:, hc, gc * P : (gc + 1) * P],
                            tb[:, hc, :],
                            start=(hc == 0), stop=(hc == NCH - 1),
                        )
                tanh_t = ptan.tile([P, NCH, TS], BF16)
                for gc in range(NCH):
                    nc.scalar.activation(
                        tanh_t[:, gc, :], pt_ps[:, gc, :], AF.Tanh,
                        bias=dpt_sb[:, gc, b : b + 1],
                    )
                tanhs[k] = tanh_t

            def energy_group(g):
                """Col-packed energy matmuls for tiles Gg..Gg+G-1 (one batch)."""
                e4 = ppe.tile([P, TS], F32, tag="e4", name="e4")
                for gc in range(NCH):
                    for j in range(G):
                        k = g * G + j
                        nc.tensor.matmul(
                            e4[32 * j : 32 * j + 1, :],
                            v_sb[:, gc : gc + 1], tanhs[k][:, gc, :],
                            start=(gc == 0), stop=(gc == NCH - 1),
                            tile_position=(0, 32 * j),
                        )
                e4s[g] = e4

            def back(k):
                """exp + broadcast + ctx accumulation for tile k."""
                b, t = divmod(k, nt)
                g, j = divmod(k, G)
                e4 = e4s[g]
                w_row = w_rows[b]
                nc.scalar.activation(
                    w_row[:, t, :], e4[32 * j : 32 * j + 1, :], AF.Exp,
                    accum_out=wsum[:, b, t : t + 1],
                )
                wb_sb = pwb.tile([P, TS], F32)
                nc.gpsimd.partition_broadcast(wb_sb[:], w_row[:, t, :])

                tb = tbs.pop(k)
                for hc in range(NCH):
                    nc.vector.scalar_tensor_tensor(
                        out=tb[:, hc, :], in0=tb[:, hc, :], scalar=1.0,
                        in1=wb_sb[:],
                        op0=mybir.AluOpType.mult, op1=mybir.AluOpType.mult,
                        accum_out=part_ts[b][:, hc, t : t + 1],
                    )
                tfs.pop(k, None)
                tanhs.pop(k, None)

            def endb(b):
                part_t = part_ts.pop(b)
                w_row = w_rows.pop(b)
                bsum = psml.tile([1, 1], F32, tag="bsum")
                nc.vector.reduce_sum(bsum[:], wsum[:, b, :], axis=mybir.AxisListType.X)
                inv_sb = psml.tile([1, 1], F32, tag="inv")
                nc.vector.reciprocal(inv_sb[:], bsum[:])

                flat = w_row[0:1, :, :].rearrange("p a b -> p (a b)")
                nc.vector.tensor_scalar(
                    out=flat, in0=flat,
                    scalar1=inv_sb[:], scalar2=None, op0=mybir.AluOpType.mult,
                )
                nc.sync.dma_start(out=attn_d[b : b + 1, :], in_=flat)

                inv_b = psml.tile([P, 1], F32, tag="invb")
                nc.gpsimd.partition_broadcast(inv_b[:], inv_sb[:])
                ctx_red = psml.tile([P, NCH], F32, tag="ctxred")
                nc.vector.reduce_sum(ctx_red[:], part_t[:], axis=mybir.AxisListType.X)
                ctx_sb = psml.tile([P, NCH], F32, tag="ctxo")
                nc.vector.tensor_scalar(
                    out=ctx_sb[:], in0=ctx_red[:], scalar1=inv_b[:],
                    scalar2=None, op0=mybir.AluOpType.mult,
                )
                nc.sync.dma_start(out=ctx_d[b].rearrange("(c p) -> p c", p=P), in_=ctx_sb[:])

            # pipeline: loads/casts run PL tiles ahead; energy group g is
            # emitted after the P-matmuls of group g+1, so neither PE nor
            # DVE ever stalls behind a ctx block.
            ngroups = total // G

            def process_group(g):
                energy_group(g)
                for kk in range(g * G, (g + 1) * G):
                    back(kk)
                if (g + 1) * G % nt == 0:
                    endb((g * G) // nt)

            PL = 4
            for k in range(total + PL):
                if k < total:
                    loadcast(k)
                kp = k - PL
                if kp >= 0:
                    pmmtanh(kp)
                    if kp % G == G - 1 and kp // G >= 1:
                        process_group(kp // G - 1)
            process_group(ngroups - 1)

    return nc


_CACHE = {}


def _get_nc(s_len):
    if s_len not in _CACHE:
        nc = bacc.Bacc("TRN2", target_bir_lowering=False, debug=False)
        build(nc, s_len)
        nc.compile()
        _CACHE[s_len] = nc
    return _CACHE[s_len]


def _prep_inputs(decoder_hidden, encoder_outputs, W1, W2, v):
    """Host-side shard: batch across 8 cores; encT layout (b, h, s) per core."""
    s_len = encoder_outputs.shape[0]
    w1t = np.ascontiguousarray(np.asarray(W1, dtype=np.float32).T)
    w2t = np.ascontiguousarray(np.asarray(W2, dtype=np.float32).T)
    v2d = np.ascontiguousarray(np.asarray(v, dtype=np.float32).reshape(1, H))
    enc = np.asarray(encoder_outputs, dtype=np.float32)
    dec = np.asarray(decoder_hidden, dtype=np.float32)
    in_maps = []
    for c in range(8):
        bsl = slice(c * NB, (c + 1) * NB)
        enc_c = np.ascontiguousarray(enc[:, bsl, :].transpose(1, 2, 0))
        dect_c = np.ascontiguousarray(dec[bsl, :].T)
        in_maps.append(
            {"enc": enc_c, "dect": dect_c, "w1t": w1t, "w2t": w2t, "v2d": v2d}
        )
    return in_maps, s_len


def kernel(decoder_hidden, encoder_outputs, W1, W2, v):
    global LAST_RESULTS
    in_maps, s_len = _prep_inputs(decoder_hidden, encoder_outputs, W1, W2, v)
    nc = _get_nc(s_len)
    res = run_bass_kernel_spmd(nc, in_maps, core_ids=list(range(8)), trace=TRACE)
    LAST_RESULTS = res
    B = 8 * NB
    context = np.empty((B, H), dtype=np.float32)
    attn = np.empty((B, s_len), dtype=np.float32)
    for c in range(8):
        bsl = slice(c * NB, (c + 1) * NB)
        context[bsl] = res.results[c]["ctx_out"]
        attn[bsl] = res.results[c]["attn_out"]
    return (context, attn)


# revision 23
# speedup vs baseline: 1.7465x; 1.1029x over previous
"""Bahdanau attention on 8 Trainium2 NeuronCores (Bass/Tile).

Problem (per reference):
  decoder_hidden (64, 512) f32, encoder_outputs (4096, 64, 512) f32,
  W1 (512,512), W2 (512,512), v (512,)
  dec_proj = decoder_hidden @ W1.T                       (B, H)
  enc_proj = einsum('bsh,gh->bsg', enc, W2)              (B, S, H)
  energy   = tanh(dec_proj[:,None,:] + enc_proj) @ v     (B, S)
  attn     = softmax(energy, axis=1)                     (B, S)
  context  = einsum('bs,bsh->bh', attn, enc)             (B, H)
  returns (context, attn)

Sharding: batch (64) split across 8 cores -> 8 batches/core; W1/W2/v
replicated. encoder_outputs is resharded host-side to (b, h, s) layout per
core so the contraction dim h lands on SBUF partitions; the kernel makes a
single pass over the 64 MB/core stream.

Per 512-column s-tile (one DMA of [128p, 4hc, 512s] f32, 2 KB rows):
  cast f32->bf16 (DVE)
  PT[g,s] = W2T-chunk.T @ encT-chunk   16 matmuls into one 4-bank PSUM tile
  tanh(PT + dec_projT[g,b])            4 ACT ops, bias folded per-partition
  energy = v.T @ tanhPT                4 matmuls -> psum [1, 512]
  w = exp(energy) (+ running sum via ACT accum), cast w bf16 (DVE)
  broadcast w across partitions (GPSIMD)
  ctx partials += encT * w             4 DVE scalar_tensor_tensor accums
End of batch: reduce partials, softmax-normalize, DMA outputs.
"""

import numpy as np
import ml_dtypes
from collections import deque

import concourse.bacc as bacc
import concourse.tile as tile
import concourse.mybir as mybir
import concourse.bass_isa as bass_isa
from concourse.bass_utils import run_bass_kernel_spmd

F32 = mybir.dt.float32
BF16 = mybir.dt.bfloat16
AF = mybir.ActivationFunctionType

NB = 8         # batches per core
H = 512
P = 128        # partitions
NCH = H // P   # h chunks (4)
TS = 512       # s columns per tile

TRACE = False
LAST_RESULTS = None


def build(nc, s_len):
    nt = s_len // TS  # s tiles per batch

    enc_d = nc.dram_tensor("enc", [NB, H, s_len], F32, kind="ExternalInput")
    dect_d = nc.dram_tensor("dect", [H, NB], F32, kind="ExternalInput")
    w1t_d = nc.dram_tensor("w1t", [H, H], F32, kind="ExternalInput")
    w2t_d = nc.dram_tensor("w2t", [H, H], F32, kind="ExternalInput")
    v_d = nc.dram_tensor("v2d", [1, H], F32, kind="ExternalInput")
    ctx_d = nc.dram_tensor("ctx_out", [NB, H], F32, kind="ExternalOutput")
    attn_d = nc.dram_tensor("attn_out", [NB, s_len], F32, kind="ExternalOutput")

    ones_d = nc.inline_tensor(np.ones((P, P), dtype=ml_dtypes.bfloat16), name="onespp")
    nt_ = s_len // TS
    G_ = min(4, nt_)
    mask_np = np.zeros((P, 1), dtype=np.float32)
    for j in range(G_):
        mask_np[32 * j, 0] = 1.0
    mask_d = nc.inline_tensor(mask_np, name="maskg")

    # persistent SBUF
    w2t_bf = nc.alloc_sbuf_tensor("w2t_bf", [P, NCH, H], BF16)   # [h, hc, g]
    ones_sb = nc.alloc_sbuf_tensor("ones_sb", [P, P], BF16)
    mask_sb = nc.alloc_sbuf_tensor("mask_sb", [P, 1], F32)
    v_sb = nc.alloc_sbuf_tensor("v_sb", [P, NCH], BF16)          # v chunks [g, gc]
    v32_sb = nc.alloc_sbuf_tensor("v32_sb", [P, NCH, 32], BF16)  # v padded for col-pack
    dpt_sb = nc.alloc_sbuf_tensor("dpt_sb", [P, NCH, NB], F32)   # dec_projT [g, gc, b]

    with tile.TileContext(nc) as tc:
        # ---------------- prologue ----------------
        with (
            tc.tile_pool(name="pro", bufs=1) as pro,
            tc.tile_pool(name="prop", bufs=1, space="PSUM") as prop,
        ):
            w2t_f = pro.tile([P, NCH, H], F32)
            nc.sync.dma_start(out=w2t_f[:], in_=w2t_d.ap().rearrange("(c p) g -> p c g", p=P))
            nc.scalar.copy(w2t_bf[:], w2t_f[:])

            nc.sync.dma_start(out=ones_sb[:], in_=ones_d[:])
            nc.sync.dma_start(out=mask_sb[:], in_=mask_d[:])

            v_f = pro.tile([P, NCH], F32)
            # v[g] -> [g % 128, g // 128]
            nc.sync.dma_start(out=v_f[:], in_=v_d.ap().rearrange("o (c p) -> p (o c)", p=P))
            nc.vector.tensor_copy(v_sb[:], v_f[:])
            nc.vector.memset(v32_sb[:], 0.0)
            for gc in range(NCH):
                nc.vector.tensor_copy(v32_sb[:, gc, 0:1], v_sb[:, gc : gc + 1])

            w1t_f = pro.tile([P, NCH, H], F32)
            nc.sync.dma_start(out=w1t_f[:], in_=w1t_d.ap().rearrange("(c p) g -> p c g", p=P))
            w1t_bf = pro.tile([P, NCH, H], BF16)
            nc.scalar.copy(w1t_bf[:], w1t_f[:])

            dect_f = pro.tile([P, NCH, NB], F32)
            nc.sync.dma_start(out=dect_f[:], in_=dect_d.ap().rearrange("(c p) b -> p c b", p=P))
            dect_bf = pro.tile([P, NCH, NB], BF16)
            nc.scalar.copy(dect_bf[:], dect_f[:])

            # dec_projT[g, b] = sum_h2 W1[g, h2] dec[b, h2]
            dp_ps = prop.tile([P, NCH, NB], F32)
            for gc in range(NCH):
                for hc in range(NCH):
                    nc.tensor.matmul(
                        dp_ps[:, gc, :],
                        w1t_bf[:, hc, gc * P : (gc + 1) * P],
                        dect_bf[:, hc, :],
                        start=(hc == 0), stop=(hc == NCH - 1),
                    )
            nc.scalar.copy(dpt_sb[:], dp_ps[:])

        # ---------------- main loop (software-pipelined emission) ----------------
        enc_r = [enc_d[b].rearrange("(c p) s -> p c s", p=P) for b in range(NB)]
        G = min(4, nt)  # energy col-pack group size
        assert nt % G == 0
        total = NB * nt

        with (
            tc.tile_pool(name="pio", bufs=7) as pio,
            tc.tile_pool(name="pbf", bufs=14) as pbf,
            tc.tile_pool(name="ptan", bufs=10) as ptan,
            tc.tile_pool(name="pw4", bufs=4) as pw4,
            tc.tile_pool(name="pacc", bufs=4) as pacc,
            tc.tile_pool(name="ppart", bufs=2) as ppart,
            tc.tile_pool(name="psml", bufs=2) as psml,
            tc.tile_pool(name="ppP", bufs=1, space="PSUM") as ppP,
            tc.tile_pool(name="ppe", bufs=1, space="PSUM") as ppe,
            tc.tile_pool(name="ppsum", bufs=1, space="PSUM") as ppsum,
            tc.tile_pool(name="ppb", bufs=2, space="PSUM") as ppb,
        ):
            pending = deque()
            tfs = {}      # k -> f32 enc tile
            tbs = {}      # k -> bf16 enc tile
            tanhs = {}    # k -> tanh tile
            part_ts = {}  # b -> ctx partials
            w4s = {}      # group -> exp(energy) rows [128, TS] (rows 32j)
            accws = {}    # group -> per-row sums [128, 1] (rows 32j)

            def loadcast(k):
                """DMA + cast for tile k (keeps DVE casts ahead of ctx blocks)."""
                b, t = divmod(k, nt)
                if t == 0:
                    part_ts[b] = ppart.tile([P, NCH, nt], F32, tag="part", name="part")
                tf = pio.tile([P, NCH, TS], F32)
                nc.sync.dma_start(out=tf[:], in_=enc_r[b][:, :, t * TS : (t + 1) * TS])
                tb = pbf.tile([P, NCH, TS], BF16)
                nc.vector.tensor_copy(tb[:], tf[:])
                tfs[k] = tf
                tbs[k] = tb

            def pmmtanh(k):
                b, t = divmod(k, nt)
                tb = tbs[k]
                pt_ps = ppP.tile([P, NCH, TS], F32)
                for gc in range(NCH):
                    for hc in range(NCH):
                        nc.tensor.matmul(
                            pt_ps[:, gc, :],
                            w2t_bf[:, hc, gc * P : (gc + 1) * P],
                            tb[:, hc, :],
                            start=(hc == 0), stop=(hc == NCH - 1),
                        )
                tanh_t = ptan.tile([P, NCH, TS], BF16)
                for gc in range(NCH):
                    nc.scalar.activation(
                        tanh_t[:, gc, :], pt_ps[:, gc, :], AF.Tanh,
                        bias=dpt_sb[:, gc, b : b + 1],
                    )
                tanhs[k] = tanh_t

            def process_energy(g):
                """Col-packed energy matmuls + one packed exp + PE broadcast +
                ctx accumulation for tiles Gg..Gg+G-1 (all same batch)."""
                b = (g * G) // nt
                e4 = ppe.tile([P, TS], F32, tag="e4", name="e4")
                for j in range(G):
                    k = g * G + j
                    for gc in range(NCH):
                        nc.tensor.matmul(
                            e4[32 * j : 32 * j + 32, :],
                            v32_sb[:, gc, :], tanhs[k][:, gc, :],
                            start=(gc == 0), stop=(gc == NCH - 1),
                            tile_position=(0, 32 * j),
                        )
                # one exp for the whole group; per-partition accum gives the
                # softmax partial sums on rows 32j for free
                w4 = pw4.tile([P, TS], F32, tag="w4", name="w4")
                accw = pacc.tile([P, 1], F32, tag="accw", name="accw")
                GG = 32 * G
                nc.scalar.activation(w4[0:GG, :], e4[0:GG, :], AF.Exp,
                                     accum_out=accw[0:GG, :])
                w4b = pw4.tile([P, TS], BF16, tag="w4b", name="w4b")
                nc.vector.tensor_copy(w4b[0:GG, :], w4[0:GG, :])
                w4s[g] = w4
                accws[g] = accw

                for j in range(G):
                    pending.append((g * G + j, w4b, j))

            def endb(b):
                part_t = part_ts.pop(b)
                gpb = nt // G  # groups per batch
                g0 = b * gpb
                GG = 32 * G
                acc = accws[g0]
                if gpb > 1:
                    acc_t = psml.tile([P, 1], F32, tag="acct")
                    nc.vector.tensor_tensor(
                        out=acc_t[0:GG, :], in0=accws[g0][0:GG, :],
                        in1=accws[g0 + 1][0:GG, :], op=mybir.AluOpType.add)
                    for g in range(2, gpb):
                        nc.vector.tensor_tensor(
                            out=acc_t[0:GG, :], in0=acc_t[0:GG, :],
                            in1=accws[g0 + g][0:GG, :], op=mybir.AluOpType.add)
                    acc = acc_t
                # sum of rows {32j} only via masked fp32 matmul
                sum_ps = ppsum.tile([1, 1], F32, tag="sum", name="sum")
                nc.tensor.matmul(sum_ps[:], acc[0:GG, :], mask_sb[0:GG, :],
                                 start=True, stop=True)
                inv1 = psml.tile([1, 1], F32, tag="inv1")
                nc.vector.reciprocal(inv1[:], sum_ps[:])
                inv_b = psml.tile([P, 1], F32, tag="invb")
                nc.gpsimd.partition_broadcast(inv_b[:], inv1[:])

                for g in range(g0, g0 + gpb):
                    w4 = w4s.pop(g)
                    nc.vector.tensor_scalar(
                        out=w4[0:GG, :], in0=w4[0:GG, :],
                        scalar1=inv_b[0:GG, :], scalar2=None,
                        op0=mybir.AluOpType.mult,
                    )
                    for j in range(G):
                        s0 = (g * G + j) % nt * TS
                        nc.sync.dma_start(
                            out=attn_d[b : b + 1, s0 : s0 + TS],
                            in_=w4[32 * j : 32 * j + 1, :])
                    accws.pop(g, None)

                ctx_red = psml.tile([P, NCH], F32, tag="ctxred")
                nc.vector.reduce_sum(ctx_red[:], part_t[:], axis=mybir.AxisListType.X)
                ctx_sb = psml.tile([P, NCH], F32, tag="ctxo")
                nc.vector.tensor_scalar(
                    out=ctx_sb[:], in0=ctx_red[:], scalar1=inv_b[:],
                    scalar2=None, op0=mybir.AluOpType.mult,
                )
                nc.sync.dma_start(out=ctx_d[b].rearrange("(c p) -> p c", p=P), in_=ctx_sb[:])

            def do_pending(n):
                """Emit wb broadcast-MM + ctx stt block for up to n pending
                tiles (spread across iterations so PE never stalls on the
                wb PSUM WAR against in-flight ctx blocks)."""
                for _ in range(min(n, len(pending))):
                    k, w4b, j = pending.popleft()
                    b, t = divmod(k, nt)
                    wb_ps = ppb.tile([P, TS], F32, tag="wb", name="wb")
                    nc.tensor.matmul(
                        wb_ps[:], ones_sb[32 * j : 32 * j + 1, :],
                        w4b[32 * j : 32 * j + 1, :],
                        start=True, stop=True, tile_position=(32 * j, 0),
                    )
                    tb = tbs.pop(k)
                    for hc in range(NCH):
                        nc.vector.scalar_tensor_tensor(
                            out=tb[:, hc, :], in0=tb[:, hc, :], scalar=1.0,
                            in1=wb_ps[:],
                            op0=mybir.AluOpType.mult, op1=mybir.AluOpType.mult,
                            accum_out=part_ts[b][:, hc, t : t + 1],
                        )
                    tfs.pop(k, None)
                    tanhs.pop(k, None)
                    if t == nt - 1:
                        endb(b)

            # pipeline: loads/casts run PL tiles ahead; energy group g is
            # emitted after the P-matmuls of group g+1; ctx blocks trail one
            # tile per iteration so no engine stalls behind them.
            ngroups = total // G
            PL = 4
            for k in range(total + PL):
                if k < total:
                    loadcast(k)
                kp = k - PL
                if kp >= 0:
                    pmmtanh(kp)
                    do_pending(1)
                    if kp % G == G - 1 and kp // G >= 1:
                        process_energy(kp // G - 1)
            process_energy(ngroups - 1)
            do_pending(len(pending) + G)

    return nc


_CACHE = {}


def _get_nc(s_len):
    if s_len not in _CACHE:
        nc = bacc.Bacc("TRN2", target_bir_lowering=False, debug=False)
        build(nc, s_len)
        nc.compile()
        _CACHE[s_len] = nc
    return _CACHE[s_len]


def _prep_inputs(decoder_hidden, encoder_outputs, W1, W2, v):
    """Host-side shard: batch across 8 cores; encT layout (b, h, s) per core."""
    s_len = encoder_outputs.shape[0]
    w1t = np.ascontiguousarray(np.asarray(W1, dtype=np.float32).T)
    w2t = np.ascontiguousarray(np.asarray(W2, dtype=np.float32).T)
    v2d = np.ascontiguousarray(np.asarray(v, dtype=np.float32).reshape(1, H))
    enc = np.asarray(encoder_outputs, dtype=np.float32)
    dec = np.asarray(decoder_hidden, dtype=np.float32)
    in_maps = []
    for c in range(8):
        bsl = slice(c * NB, (c + 1) * NB)
        enc_c = np.ascontiguousarray(enc[:, bsl, :].transpose(1, 2, 0))
        dect_c = np.ascontiguousarray(dec[bsl, :].T)
        in_maps.append(
            {"enc": enc_c, "dect": dect_c, "w1t": w1t, "w2t": w2t, "v2d": v2d}
        )
    return in_maps, s_len


def kernel(decoder_hidden, encoder_outputs, W1, W2, v):
    global LAST_RESULTS
    in_maps, s_len = _prep_inputs(decoder_hidden, encoder_outputs, W1, W2, v)
    nc = _get_nc(s_len)
    res = run_bass_kernel_spmd(nc, in_maps, core_ids=list(range(8)), trace=TRACE)
    LAST_RESULTS = res
    B = 8 * NB
    context = np.empty((B, H), dtype=np.float32)
    attn = np.empty((B, s_len), dtype=np.float32)
    for c in range(8):
        bsl = slice(c * NB, (c + 1) * NB)
        context[bsl] = res.results[c]["ctx_out"]
        attn[bsl] = res.results[c]["attn_out"]
    return (context, attn)
